# revision 51
# baseline (speedup 1.0000x reference)
"""Trainium2 Bass kernel for nn_ByteEncoder (multi-scale conv stem + per-channel LRU).

Sharding: 8 cores = (batch b in 0..3) x (time-half h in 0..1). Each core runs an
identical SPMD program over raw steps [t0-512, t0+4096) (t0 = h*4096), i.e. a
128-scan-step warmup plus its 1024 output scan steps. The warmup region is
masked to zero for h=0 cores (reference scan starts at state 0) and uses real
left-context for h=1 cores (per-channel decay lambda^128 < 1e-23).

The embedding lookup is algebraically fused into the conv stem: for one-hot
inputs, conv_k(embed[x]) == sum_taps (embed @ conv_w[:,:,j])[x[t+off]], so the
stem becomes matmuls of precontracted [256-vocab x 256-ch] tables against
one-hot columns built on-chip (iota + is_equal).

v2: single fused pipeline, fully SBUF-resident (no DRAM bounce of h_multi or
h_down), all matmul operands in bf16 (full PE rate at any row count, half the
weight DMA), software-pipelined so stem(t+2) covers down(t)'s dependencies and
the b-proj/scan groups interleave into the stem/down PE stream. Elementwise
ops that are identities for the given inputs (zero biases, unit LN weights)
are skipped at build time; a general fallback path applies them.
"""
import numpy as np

import concourse.bass as bass
import concourse.tile as tile
from concourse import mybir, bacc
from concourse.bass_utils import run_bass_kernel_spmd
from concourse.masks import make_identity

P = 128
D = 1024
B = 4
T = 8192
VOCAB = 256
SENTINEL = 512.0  # out-of-range token -> one-hot col is all zero

W_SCAN = 32             # warmup scan steps (lam_max^32 ~ 1.5e-6, far below tol)
S_LOC = 1024 + W_SCAN   # scan steps computed per core (chunk 0 = warmup)
T_LOC = 4 * S_LOC       # raw steps per core (4224)
X_LOC = T_LOC + 8       # x slice incl conv halo (left 4, right 3, +1 pad)
N_TT = 9                # T-tiles: tile 0 = 128-raw warmup, tiles 1..8 = 512
N_CH = 9                # scan chunks: chunk 0 = 32 steps, 1..8 = 128 steps


def tile_cols(tt):       # raw positions in stem tile tt
    return 4 * W_SCAN if tt == 0 else 512


def tile_x0(tt):         # x_loc offset of stem tile tt
    return 0 if tt == 0 else 4 * W_SCAN + 512 * (tt - 1)


def chunk_col(c):        # first scan column of chunk c in hsT/h_all
    return 0 if c == 0 else W_SCAN + 128 * (c - 1)

f32 = mybir.dt.float32
bf16 = mybir.dt.bfloat16
f8 = mybir.dt.float8e4
AF = mybir.ActivationFunctionType
OP = mybir.AluOpType
DR = mybir.MatmulPerfMode.DoubleRow

# (kernel_size, pad); tap offset = j - pad
CONVS = [(1, 0), (2, 1), (4, 2), (8, 4)]
TAPS = []  # (conv_id, j, off)
for ci, (K, pad) in enumerate(CONVS):
    for j in range(K):
        TAPS.append((ci, j, j - pad))
N_TAPS = len(TAPS)  # 15
TAPS_OF_CONV = [[kk for kk, (ci, _, _) in enumerate(TAPS) if ci == c] for c in range(4)]

# channel-block order inside a stem tile: half-0 blocks first (so the first
# stem-table DMA chunk covers the early matmuls), big blocks interleaved with
# small ones so the gelu of a small block has a long window before its PSUM
# buf is reused
CC_ORDER = [6, 0, 4, 2, 7, 1, 5, 3]

# scan groups as (first_col, width) over the S_LOC scan columns
GROUPS = [(0, W_SCAN + 256), (W_SCAN + 256, 384), (W_SCAN + 640, 384)]

_CACHE = {}


def _build(flags):
    """flags: (convb0, downb0, slnw1, bb20, slncb0, lruw1, lrub0) — True means
    the corresponding elementwise op is an identity and is skipped."""
    convb0, downb0, slnw1, bb20, slncb0, lruw1, lrub0 = flags
    nc = bacc.Bacc()

    x_d = nc.declare_dram_parameter("x_rep", [N_TT, P, 520], bf16, isOutput=False)
    mask_d = nc.declare_dram_parameter("mask", [S_LOC], f32, isOutput=False)
    # [vc, p, half, tap, 128]: channel-halves contiguous per partition so the
    # half-0 DMA covers a clean flat interval (no false deps on the half-1 DMA)
    stem_d = nc.declare_dram_parameter("stem_w", [2, P, 2, N_TAPS, 128], bf16, isOutput=False)
    convb_d = nc.declare_dram_parameter("convb", [P, 8], f32, isOutput=False)
    dw_d = nc.declare_dram_parameter("down_wt", [P, 4, 8, D], bf16, isOutput=False)
    downb_d = nc.declare_dram_parameter("down_b", [D], f32, isOutput=False)
    bw_d = nc.declare_dram_parameter("b_wt", [P, 8, D], f8, isOutput=False)
    bb2_d = nc.declare_dram_parameter("bb2", [P, 8], f32, isOutput=False)
    cw_d = nc.declare_dram_parameter("c_wt", [P, 8, D], bf16, isOutput=False)
    slnw_d = nc.declare_dram_parameter("slnw", [D], f32, isOutput=False)
    slncb_d = nc.declare_dram_parameter("slncb", [D], f32, isOutput=False)
    lruw_d = nc.declare_dram_parameter("lruw", [D], f32, isOutput=False)
    lrub_d = nc.declare_dram_parameter("lrub", [D], f32, isOutput=False)
    lam_d = nc.declare_dram_parameter("lam_ct", [P, 8], f32, isOutput=False)

    out_d = nc.declare_dram_parameter("out", [1024, D], f32, isOutput=True)

    with tile.TileContext(nc) as tc:
        with tc.tile_pool(name="glob", bufs=1) as glob:
            MW = W_SCAN + 256  # group-0 width (warmup + chunks 1-2)
            lam_sb = glob.tile([P, 8], f32, name="lam_sb")
            mask_rep = glob.tile([P, MW], f32, name="mask_rep")
            eps_sb = glob.tile([P, 1], f32, name="eps_sb")
            nc.vector.memset(eps_sb[:], 1e-5)
            ident = glob.tile([P, P], f32, name="ident")
            make_identity(nc, ident)
            ident_b = glob.tile([P, P], bf16, name="ident_b")
            nc.scalar.copy(ident_b[:], ident[:])
            io0 = glob.tile([P, 1], f32, name="io0")
            io1 = glob.tile([P, 1], f32, name="io1")
            nc.gpsimd.iota(io0[:], pattern=[[0, 1]], base=0, channel_multiplier=1,
                           allow_small_or_imprecise_dtypes=True)
            nc.gpsimd.iota(io1[:], pattern=[[0, 1]], base=128, channel_multiplier=1,
                           allow_small_or_imprecise_dtypes=True)
            convb_sb = None
            if not convb0:
                convb_sb = glob.tile([P, 8], f32, name="convb_sb")
                nc.sync.dma_start(convb_sb[:], convb_d[:])
            bb2_sb = None
            if not bb20:
                bb2_sb = glob.tile([P, 8], f32, name="bb2_sb")
                nc.sync.dma_start(bb2_sb[:], bb2_d[:])
            downb_rep = None
            if not downb0:
                downb_rep = glob.tile([P, D], f32, name="downb_rep")
                nc.sync.dma_start(downb_rep[:],
                                  downb_d[:][None, :].to_broadcast([P, D]))
            slnw_rep = None
            if not slnw1:
                slnw_rep = glob.tile([P, D], f32, name="slnw_rep")
                nc.sync.dma_start(slnw_rep[:],
                                  slnw_d[:][None, :].to_broadcast([P, D]))

            # persistent across phases
            z_all = glob.tile([P, N_CH, D], bf16, name="z_all")
            # hsT/bw in fp8e4: the b-proj runs as DoubleRow matmuls (K=256 per
            # instruction, ec-pairs adjacent in both layouts). z stays bf16 for
            # the residual; only the b-proj input is quantized (~1e-2 end2end,
            # gate is 2e-2).
            hsT = glob.tile([P, 8, S_LOC], f8, name="hsT")
            h_all = glob.tile([P, 8, S_LOC], bf16, name="h_all")
            bw_sb = glob.tile([P, 8, D], f8, name="bw_sb")
            # first e-half of the c-proj weights, prefetched late in phase A
            # (the full 2.1MB does not fit alongside the phase-A pools)
            cw_h0 = glob.tile([P, 8, 512], bf16, name="cw_h0")

            def ln_from_psums(pools, ps0, ps1_, z_out, rows):
                """LayerNorm over D from two [rows,512] psum halves -> z (bf16)."""
                p1t = pools
                stats = p1t.tile([P, 2, 6], f32, name="stats", bufs=2)
                nc.vector.bn_stats(out=stats[0:rows, 0, :], in_=ps0)
                nc.vector.bn_stats(out=stats[0:rows, 1, :], in_=ps1_)
                mv = p1t.tile([P, 2], f32, name="mv", bufs=2)
                nc.vector.bn_aggr(out=mv[0:rows, :], in_=stats[0:rows, :, :])
                rstd = p1t.tile([P, 1], f32, name="rstd", bufs=2)
                nc.scalar.activation(rstd[0:rows, :], mv[0:rows, 1:2], AF.Sqrt,
                                     bias=eps_sb[0:rows, :])
                nc.vector.reciprocal(rstd[0:rows, :], rstd[0:rows, :])
                nc.vector.tensor_scalar(out=z_out[0:rows, 0:512], in0=ps0,
                                        scalar1=mv[0:rows, 0:1],
                                        scalar2=rstd[0:rows, :],
                                        op0=OP.subtract, op1=OP.mult)
                nc.vector.tensor_scalar(out=z_out[0:rows, 512:1024], in0=ps1_,
                                        scalar1=mv[0:rows, 0:1],
                                        scalar2=rstd[0:rows, :],
                                        op0=OP.subtract, op1=OP.mult)
                if not slnw1:
                    nc.gpsimd.tensor_tensor(out=z_out[0:rows, :],
                                            in0=z_out[0:rows, :],
                                            in1=slnw_rep[0:rows, :], op=OP.mult)

            def bproj_scan(p1t, psb_pool, col0, W):
                for dc in range(8):
                    psb = psb_pool.tile([P, 512], f32, name="psb", tag="psb")
                    for ecp in range(4):
                        nc.tensor.matmul(
                            psb[:, :W],
                            bw_sb[:, 2 * ecp: 2 * ecp + 2, dc * 128:(dc + 1) * 128],
                            hsT[:, 2 * ecp: 2 * ecp + 2, col0: col0 + W],
                            start=(ecp == 0), stop=(ecp == 3),
                            perf_mode=DR)
                    data1 = psb[:, :W]
                    if (col0 == 0) or (not bb20):
                        vals = p1t.tile([P, 512], f32, name="vals", bufs=2)
                        if not bb20:
                            nc.vector.tensor_scalar(out=vals[:, :W], in0=psb[:, :W],
                                                    scalar1=bb2_sb[:, dc:dc + 1],
                                                    scalar2=None, op0=OP.add)
                            if col0 == 0:
                                nc.vector.tensor_tensor(
                                    out=vals[:, :W], in0=vals[:, :W],
                                    in1=mask_rep[:, :W], op=OP.mult)
                        else:
                            nc.vector.tensor_tensor(
                                out=vals[:, :W], in0=psb[:, :W],
                                in1=mask_rep[:, :W], op=OP.mult)
                        data1 = vals[:, :W]
                    init = (0.0 if col0 == 0
                            else h_all[:, dc, col0 - 1: col0])
                    nc.vector.tensor_tensor_scan(
                        out=h_all[:, dc, col0: col0 + W],
                        data0=lam_sb[:, dc:dc + 1].to_broadcast([P, W]),
                        data1=data1,
                        initial=init, op0=OP.mult, op1=OP.add)

            # ---------------- Phase A: fused stem + down-conv + LN + z^T ----
            with tc.tile_pool(name="paw", bufs=1) as paw, \
                 tc.tile_pool(name="pat", bufs=2) as pat, \
                 tc.tile_pool(name="ps1", bufs=2, space="PSUM") as ps1, \
                 tc.tile_pool(name="psd", bufs=2, space="PSUM") as psd, \
                 tc.tile_pool(name="pst", bufs=2, space="PSUM") as pst, \
                 tc.tile_pool(name="psb", bufs=2, space="PSUM") as psb_pool:
                stem_sb0 = paw.tile([P, 2, N_TAPS, 128], bf16, name="stem_sb0")
                stem_sb1 = paw.tile([P, 2, N_TAPS, 128], bf16, name="stem_sb1")
                stem_sbs = (stem_sb0, stem_sb1)
                dw_sb = paw.tile([P, 4, 8, D], bf16, name="dw_sb")

                x_reps, ohs = {}, {}

                def issue_xrep(tt):
                    x_rep = pat.tile([P, 520], bf16, name="x_rep", bufs=3)
                    nc.sync.dma_start(x_rep[:], x_d[tt])
                    x_reps[tt] = x_rep

                # latency-critical first: x for tile 0, then the half-0 stem
                # tables (the first CC_ORDER blocks read only half 0)
                issue_xrep(0)
                nc.sync.dma_start(stem_sb0[:, 0], stem_d[0][:, 0])
                def build_oh(tt):
                    w8 = tile_cols(tt) + 8
                    x_rep = x_reps.pop(tt)
                    oh = pat.tile([P, 2, 520], bf16, name="oh", bufs=3)
                    nc.vector.tensor_scalar(out=oh[:, 0, 0:w8],
                                            in0=x_rep[:, 0:w8],
                                            scalar1=io0[:], scalar2=None,
                                            op0=OP.is_equal)
                    nc.vector.tensor_scalar(out=oh[:, 1, 0:w8],
                                            in0=x_rep[:, 0:w8],
                                            scalar1=io1[:], scalar2=None,
                                            op0=OP.is_equal)
                    ohs[tt] = oh

                hm_ts = {}

                def stem(tt):
                    cols = tile_cols(tt)
                    oh = ohs.pop(tt)
                    hm_t = pat.tile([P, 8, 512], bf16, name="hm_t", bufs=3)
                    hm_ts[tt] = hm_t
                    for cc in CC_ORDER:
                        ci, half = cc // 2, cc % 2
                        taps = TAPS_OF_CONV[ci]
                        ps = ps1.tile([P, 512], f32, name="ps", tag="ps")
                        n_mm = len(taps) * 2
                        i = 0
                        for vc in range(2):
                            for kk in taps:
                                off = TAPS[kk][2]
                                nc.tensor.matmul(
                                    ps[:, 0:cols],
                                    stem_sbs[vc][:, half, kk, :],
                                    oh[:, vc, 4 + off: 4 + off + cols],
                                    start=(i == 0), stop=(i == n_mm - 1))
                                i += 1
                        bias = 0.0 if convb0 else convb_sb[:, cc:cc + 1]
                        nc.scalar.activation(hm_t[:, cc, 0:cols], ps[:, 0:cols],
                                             AF.Gelu, bias=bias)

                def down_ln_warm(tt):
                    """Warmup-tile down-conv in [e, s] orientation (32-row
                    matmuls), then PE-transpose back for the standard LN."""
                    S = tile_cols(tt) // 4          # 32 scan steps
                    hm_sb = hm_ts.pop(tt)
                    hdT = pat.tile([P, 8 * S], bf16, name="hdT", bufs=1)
                    for ec in range(8):
                        psw_t = psd.tile([P, 512], f32, name="psdt", tag="psdt")
                        psw = psw_t[:, 0:S]
                        i = 0
                        for j in range(4):
                            for dc in range(8):
                                nc.tensor.matmul(
                                    psw[:],
                                    dw_sb[:, j, dc, ec * 128:(ec + 1) * 128],
                                    hm_sb[:, dc, j:4 * S:4],
                                    start=(i == 0), stop=(i == 31))
                                i += 1
                        nc.scalar.copy(hdT[:, ec * S:(ec + 1) * S], psw[:])
                    hd0 = pat.tile([P, D], bf16, name="hd0", bufs=1)
                    for ec in range(8):
                        pt = pst.tile([P, P], bf16, name="pt", tag="pt")
                        nc.tensor.transpose(
                            pt[0:S, :], hdT[:, ec * S:(ec + 1) * S],
                            ident_b[:])
                        nc.scalar.copy(hd0[0:S, ec * 128:(ec + 1) * 128],
                                       pt[0:S, :])
                    p0, p1_ = hd0[0:S, 0:512], hd0[0:S, 512:1024]
                    if not downb0:
                        t0 = pat.tile([P, D], f32, name="hd_t", bufs=2)
                        nc.vector.tensor_tensor(out=t0[0:S, :], in0=hd0[0:S, :],
                                                in1=downb_rep[0:S, :], op=OP.add)
                        p0, p1_ = t0[0:S, 0:512], t0[0:S, 512:1024]
                    ln_from_psums(pat, p0, p1_, z_all[:, tt, :], S)

                def down_ln(tt):
                    if tt == 0:
                        return down_ln_warm(tt)
                    cols = tile_cols(tt)
                    rows = cols // 4
                    hm_sb = hm_ts.pop(tt)
                    pss = []
                    for eh in range(2):
                        ps = psd.tile([P, 512], f32, name="psdt", tag="psdt")
                        i = 0
                        for j in range(4):
                            for dc in range(8):
                                nc.tensor.matmul(
                                    ps[0:rows, :],
                                    hm_sb[:, dc, j:cols:4],
                                    dw_sb[:, j, dc, eh * 512:(eh + 1) * 512],
                                    start=(i == 0), stop=(i == 31))
                                i += 1
                        pss.append(ps)
                    p0, p1_ = pss[0][0:rows, :], pss[1][0:rows, :]
                    if not downb0:
                        t0 = pat.tile([P, D], f32, name="hd_t", bufs=2)
                        nc.vector.tensor_tensor(out=t0[0:rows, 0:512], in0=p0,
                                                in1=downb_rep[0:rows, 0:512],
                                                op=OP.add)
                        nc.vector.tensor_tensor(out=t0[0:rows, 512:1024], in0=p1_,
                                                in1=downb_rep[0:rows, 512:1024],
                                                op=OP.add)
                        p0, p1_ = t0[0:rows, 0:512], t0[0:rows, 512:1024]
                    ln_from_psums(pat, p0, p1_, z_all[:, tt, :], rows)

                def transp(tt):
                    rows = tile_cols(tt) // 4
                    col = chunk_col(tt)
                    for ec in range(8):
                        pt = pst.tile([P, P], bf16, name="pt", tag="pt")
                        nc.tensor.transpose(
                            pt[:, 0:rows],
                            z_all[0:rows, tt, ec * 128:(ec + 1) * 128],
                            ident_b[0:rows, 0:rows])
                        nc.scalar.copy(hsT[:, ec, col: col + rows], pt[:, 0:rows])

                # -- emission: 3-tile software pipeline --
                nc.sync.dma_start(stem_sb1[:, 0], stem_d[1][:, 0])
                issue_xrep(1)
                nc.sync.dma_start(stem_sb0[:, 1], stem_d[0][:, 1])
                nc.sync.dma_start(stem_sb1[:, 1], stem_d[1][:, 1])
                for tt in (2, 3, 4):
                    issue_xrep(tt)
                # small params deferred behind the latency-critical loads
                nc.sync.dma_start(lam_sb[:], lam_d[:])
                nc.sync.dma_start(mask_rep[:],
                                  mask_d[0:MW][None, :].to_broadcast([P, MW]))
                # bulk weight loads: issued before any consumer instruction
                for j in range(4):
                    for k in range(2):
                        nc.sync.dma_start(dw_sb[:, j, k * 4:(k + 1) * 4, :],
                                          dw_d[:, j, k * 4:(k + 1) * 4, :])
                nc.sync.dma_start(bw_sb[:], bw_d[:])
                # dummy transposes ramp the PE p-state while the first DMAs
                # land; nothing reads the results
                for _ in range(16):
                    ptw = pst.tile([P, P], bf16, name="pt", tag="pt")
                    nc.tensor.transpose(ptw[:], ident_b[:], ident_b[:])
                build_oh(0)
                stem(0)
                build_oh(1)
                stem(1)
                build_oh(2)
                stem(2)
                for t in range(N_TT):
                    if t + 5 < N_TT:
                        issue_xrep(t + 5)
                    if t + 3 < N_TT:
                        build_oh(t + 3)
                    down_ln(t)
                    if t + 3 < N_TT:
                        stem(t + 3)
                    if t >= 1:
                        transp(t - 1)
                    if t == 3:
                        bproj_scan(pat, psb_pool, *GROUPS[0])
                    if t == 6:
                        bproj_scan(pat, psb_pool, *GROUPS[1])
                    if t == 7:
                        nc.sync.dma_start(cw_h0[:], cw_d[:, :, 0:512])
                transp(N_TT - 1)
                bproj_scan(pat, psb_pool, *GROUPS[2])

            # ---------------- Phase C: c-proj + residual + final LN ---------
            with tc.tile_pool(name="pcw", bufs=1) as pcw, \
                 tc.tile_pool(name="pct", bufs=2) as pct, \
                 tc.tile_pool(name="psc", bufs=4, space="PSUM") as psc_pool:
                cw_h1 = pcw.tile([P, 8, 512], bf16, name="cw_h1")
                nc.sync.dma_start(cw_h1[:], cw_d[:, :, 512:1024])
                cw_halves = (cw_h0, cw_h1)
                slncb_rep = None
                if not slncb0:
                    slncb_rep = pcw.tile([P, D], f32, name="slncb_rep")
                    nc.sync.dma_start(slncb_rep[:],
                                      slncb_d[:][None, :].to_broadcast([P, D]))
                lruw_rep = lrub_rep = None
                if not lruw1:
                    lruw_rep = pcw.tile([P, D], f32, name="lruw_rep")
                    nc.sync.dma_start(lruw_rep[:],
                                      lruw_d[:][None, :].to_broadcast([P, D]))
                if not lrub0:
                    lrub_rep = pcw.tile([P, D], f32, name="lrub_rep")
                    nc.sync.dma_start(lrub_rep[:],
                                      lrub_d[:][None, :].to_broadcast([P, D]))

                for c in range(1, N_CH):
                    col = chunk_col(c)
                    pscs = []
                    for eh in range(2):
                        psc = psc_pool.tile([P, 512], f32, name="psc", tag="psc")
                        for dc in range(8):
                            nc.tensor.matmul(
                                psc[:],
                                h_all[:, dc, col: col + 128],
                                cw_halves[eh][:, dc, :],
                                start=(dc == 0), stop=False)
                        # residual: accumulate z into the c-proj psum via an
                        # identity matmul (frees the DVE, the phase-C
                        # bottleneck engine)
                        nc.tensor.matmul(
                            psc[:], ident_b[:],
                            z_all[:, c, eh * 512:(eh + 1) * 512],
                            start=False, stop=True)
                        pscs.append(psc)
                    res_h = (pscs[0][:], pscs[1][:])
                    if not slncb0:
                        res = pct.tile([P, D], f32, name="res", bufs=2)
                        nc.vector.tensor_tensor(out=res[:, 0:512], in0=pscs[0][:],
                                                in1=slncb_rep[:, 0:512], op=OP.add)
                        nc.vector.tensor_tensor(out=res[:, 512:1024],
                                                in0=pscs[1][:],
                                                in1=slncb_rep[:, 512:1024],
                                                op=OP.add)
                        res_h = (res[:, 0:512], res[:, 512:1024])
                    stats2 = pct.tile([P, 2, 6], f32, name="stats2", bufs=2)
                    nc.vector.bn_stats(out=stats2[:, 0, :], in_=res_h[0])
                    nc.vector.bn_stats(out=stats2[:, 1, :], in_=res_h[1])
                    mv2 = pct.tile([P, 2], f32, name="mv2", bufs=2)
                    nc.vector.bn_aggr(out=mv2[:], in_=stats2[:])
                    rstd2 = pct.tile([P, 1], f32, name="rstd2", bufs=2)
                    nc.scalar.activation(rstd2[:], mv2[:, 1:2], AF.Sqrt,
                                         bias=eps_sb[:])
                    nc.vector.reciprocal(rstd2[:], rstd2[:])
                    o_t = pct.tile([P, D], f32, name="o_t", bufs=2)
                    nc.vector.tensor_scalar(out=o_t[:, 0:512], in0=res_h[0],
                                            scalar1=mv2[:, 0:1],
                                            scalar2=rstd2[:],
                                            op0=OP.subtract, op1=OP.mult)
                    nc.vector.tensor_scalar(out=o_t[:, 512:1024], in0=res_h[1],
                                            scalar1=mv2[:, 0:1],
                                            scalar2=rstd2[:],
                                            op0=OP.subtract, op1=OP.mult)
                    if not lruw1:
                        nc.gpsimd.tensor_tensor(out=o_t[:], in0=o_t[:],
                                                in1=lruw_rep[:], op=OP.mult)
                    if not lrub0:
                        nc.gpsimd.tensor_tensor(out=o_t[:], in0=o_t[:],
                                                in1=lrub_rep[:], op=OP.add)
                    nc.sync.dma_start(out_d[(c - 1) * 128: c * 128, :], o_t[:])

    nc.finalize()
    return nc


def _prep_host(inputs):
    f = np.float32
    import ml_dtypes
    bf = ml_dtypes.bfloat16
    embed = np.asarray(inputs["embed"], f)
    conv_ws = [np.asarray(inputs[k], f) for k in
               ("conv1_w", "conv2_w", "conv4_w", "conv8_w")]
    conv_bs = [np.asarray(inputs[k], f) for k in
               ("conv1_b", "conv2_b", "conv4_b", "conv8_b")]
    down_w = np.asarray(inputs["down_w"], f)
    log_lam = np.asarray(inputs["log_lambda_raw"], f)
    lam = (1.0 / (1.0 + np.exp(-log_lam.astype(np.float64)))).astype(f)
    b_w = np.asarray(inputs["b_w"], f)
    c_w = np.asarray(inputs["c_w"], f)
    slnw = np.asarray(inputs["stem_ln_w"], f)
    slnb = np.asarray(inputs["stem_ln_b"], f)
    lruw = np.asarray(inputs["lru_ln_w"], f)
    lrub = np.asarray(inputs["lru_ln_b"], f)
    c_b = np.asarray(inputs["c_b"], f)
    down_b = np.asarray(inputs["down_b"], f)

    stem_w = np.empty((2, P, 2, N_TAPS, 128), bf)
    for kk, (ci, j, _off) in enumerate(TAPS):
        fused = embed @ conv_ws[ci][:, :, j].T        # [256v, 256c]
        stem_w[:, :, :, kk, :] = fused.reshape(2, P, 2, 128).astype(bf)
    convb = np.concatenate(conv_bs).reshape(8, P).T.copy()      # [p, cc]

    down_wt = (down_w.transpose(1, 2, 0)                        # [d, j, e]
               .reshape(8, P, 4, D).transpose(1, 2, 0, 3)
               .astype(bf))                                     # [p, j, dc, e]
    one_m = (1.0 - lam)
    # values[d,t] = sum_e [(1-lam_d) b_w[d,e]] zs^T[e,t] + (1-lam)(b_w@slnb + b_b)
    # (zs = z*slnw is materialized on-device; slnb folds into bb2)
    b_wt = ((b_w.T * one_m[None, :])                            # [e, d]
            .reshape(8, P, D).transpose(1, 0, 2)
            .astype(ml_dtypes.float8_e4m3fn))                   # [p, ec, d]
    bb2 = (one_m * (b_w @ slnb + np.asarray(inputs["b_b"], f))
           ).reshape(8, P).T.copy()
    c_wt = c_w.T.reshape(8, P, D).transpose(1, 0, 2).astype(bf)  # [p, dc, e]
    lam_ct = lam.reshape(8, P).T.copy()
    slncb = slnb + c_b

    flags = (
        bool(np.all(convb == 0.0)),
        bool(np.all(down_b == 0.0)),
        bool(np.all(slnw == 1.0)),
        bool(np.all(bb2 == 0.0)),
        bool(np.all(slncb == 0.0)),
        bool(np.all(lruw == 1.0)),
        bool(np.all(lrub == 0.0)),
    )

    shared = dict(
        stem_w=stem_w, convb=convb, down_wt=down_wt, down_b=down_b,
        b_wt=b_wt, bb2=bb2, c_wt=c_wt, slncb=slncb,
        slnw=slnw, lruw=lruw, lrub=lrub, lam_ct=lam_ct,
    )

    x = np.asarray(inputs["x"]).astype(np.int64)
    in_maps = []
    for core in range(8):
        b, h = core // 2, core % 2
        t0 = h * 4096
        idx = t0 - (4 * W_SCAN + 4) + np.arange(X_LOC)
        valid = (idx >= 0) & (idx < T)
        x_loc = np.full((X_LOC,), SENTINEL, bf)
        x_loc[valid] = x[b, idx[valid]].astype(bf)
        # pre-broadcast per tile on host: device-side 128-way broadcast DMAs
        # are far slower than a contiguous copy
        x_rep = np.full((N_TT, P, 520), SENTINEL, bf)
        for tt in range(N_TT):
            w8 = tile_cols(tt) + 8
            x0_ = tile_x0(tt)
            x_rep[tt, :, 0:w8] = x_loc[x0_: x0_ + w8][None, :]
        mask = np.ones((S_LOC,), f)
        if h == 0:
            mask[:W_SCAN] = 0.0
        m = dict(shared)
        m["x_rep"] = x_rep
        m["mask"] = mask
        in_maps.append(m)
    return in_maps, flags


def kernel(**inputs) -> np.ndarray:
    in_maps, flags = _prep_host(inputs)
    if flags not in _CACHE:
        _CACHE[flags] = _build(flags)
    nc = _CACHE[flags]
    res = run_bass_kernel_spmd(nc, in_maps, list(range(8)))
    out = np.empty((B, 2048, D), np.float32)
    for core in range(8):
        b, h = core // 2, core % 2
        out[b, h * 1024:(h + 1) * 1024, :] = res.results[core]["out"]
    return out


# revision 52
# speedup vs baseline: 1.2011x; 1.2011x over previous
"""Trainium2 Bass kernel for nn_ByteEncoder (multi-scale conv stem + per-channel LRU).

Sharding: 8 cores = (batch b in 0..3) x (time-half h in 0..1). Each core runs an
identical SPMD program over raw steps [t0-512, t0+4096) (t0 = h*4096), i.e. a
128-scan-step warmup plus its 1024 output scan steps. The warmup region is
masked to zero for h=0 cores (reference scan starts at state 0) and uses real
left-context for h=1 cores (per-channel decay lambda^128 < 1e-23).

The embedding lookup is algebraically fused into the conv stem: for one-hot
inputs, conv_k(embed[x]) == sum_taps (embed @ conv_w[:,:,j])[x[t+off]], so the
stem becomes matmuls of precontracted [256-vocab x 256-ch] tables against
one-hot columns built on-chip (iota + is_equal).

v2: single fused pipeline, fully SBUF-resident (no DRAM bounce of h_multi or
h_down), all matmul operands in bf16 (full PE rate at any row count, half the
weight DMA), software-pipelined so stem(t+2) covers down(t)'s dependencies and
the b-proj/scan groups interleave into the stem/down PE stream. Elementwise
ops that are identities for the given inputs (zero biases, unit LN weights)
are skipped at build time; a general fallback path applies them.
"""
import numpy as np

import concourse.bass as bass
import concourse.tile as tile
from concourse import mybir, bacc
from concourse.bass_utils import run_bass_kernel_spmd
from concourse.masks import make_identity

P = 128
D = 1024
B = 4
T = 8192
VOCAB = 256
SENTINEL = 512.0  # out-of-range token -> one-hot col is all zero

W_SCAN = 32             # warmup scan steps (lam_max^32 ~ 1.5e-6, far below tol)
S_LOC = 1024 + W_SCAN   # scan steps computed per core (chunk 0 = warmup)
T_LOC = 4 * S_LOC       # raw steps per core (4224)
X_LOC = T_LOC + 8       # x slice incl conv halo (left 4, right 3, +1 pad)
N_TT = 9                # T-tiles: tile 0 = 128-raw warmup, tiles 1..8 = 512
N_CH = 9                # scan chunks: chunk 0 = 32 steps, 1..8 = 128 steps


def tile_cols(tt):       # raw positions in stem tile tt
    return 4 * W_SCAN if tt == 0 else 512


def tile_x0(tt):         # x_loc offset of stem tile tt
    return 0 if tt == 0 else 4 * W_SCAN + 512 * (tt - 1)


def chunk_col(c):        # first scan column of chunk c in hsT/h_all
    return 0 if c == 0 else W_SCAN + 128 * (c - 1)

f32 = mybir.dt.float32
bf16 = mybir.dt.bfloat16
f8 = mybir.dt.float8e4
AF = mybir.ActivationFunctionType
OP = mybir.AluOpType
DR = mybir.MatmulPerfMode.DoubleRow

# (kernel_size, pad); tap offset = j - pad
CONVS = [(1, 0), (2, 1), (4, 2), (8, 4)]
TAPS = []  # (conv_id, j, off)
for ci, (K, pad) in enumerate(CONVS):
    for j in range(K):
        TAPS.append((ci, j, j - pad))
N_TAPS = len(TAPS)  # 15
TAPS_OF_CONV = [[kk for kk, (ci, _, _) in enumerate(TAPS) if ci == c] for c in range(4)]

# channel-block order inside a stem tile: half-0 blocks first (so the first
# stem-table DMA chunk covers the early matmuls), big blocks interleaved with
# small ones so the gelu of a small block has a long window before its PSUM
# buf is reused
CC_ORDER = [6, 0, 4, 2, 7, 1, 5, 3]

# scan groups as (first_col, width) over the S_LOC scan columns
GROUPS = [(0, W_SCAN + 256), (W_SCAN + 256, 384), (W_SCAN + 640, 384)]

_CACHE = {}


def _build(flags):
    """flags: (convb0, downb0, slnw1, bb20, slncb0, lruw1, lrub0) — True means
    the corresponding elementwise op is an identity and is skipped."""
    convb0, downb0, slnw1, bb20, slncb0, lruw1, lrub0 = flags
    nc = bacc.Bacc()

    x_d = nc.declare_dram_parameter("x_rep", [N_TT, P, 520], bf16, isOutput=False)
    mask_d = nc.declare_dram_parameter("mask", [S_LOC], f32, isOutput=False)
    # [vc, p, half, tap, 128]: channel-halves contiguous per partition so the
    # half-0 DMA covers a clean flat interval (no false deps on the half-1 DMA)
    stem_d = nc.declare_dram_parameter("stem_w", [2, P, 2, N_TAPS, 128], bf16, isOutput=False)
    convb_d = nc.declare_dram_parameter("convb", [P, 8], f32, isOutput=False)
    dw_d = nc.declare_dram_parameter("down_wt", [P, 4, 8, D], bf16, isOutput=False)
    downb_d = nc.declare_dram_parameter("down_b", [D], f32, isOutput=False)
    bw_d = nc.declare_dram_parameter("b_wt", [P, 8, D], f8, isOutput=False)
    bb2_d = nc.declare_dram_parameter("bb2", [P, 8], f32, isOutput=False)
    cw_d = nc.declare_dram_parameter("c_wt", [P, 8, D], bf16, isOutput=False)
    slnw_d = nc.declare_dram_parameter("slnw", [D], f32, isOutput=False)
    slncb_d = nc.declare_dram_parameter("slncb", [D], f32, isOutput=False)
    lruw_d = nc.declare_dram_parameter("lruw", [D], f32, isOutput=False)
    lrub_d = nc.declare_dram_parameter("lrub", [D], f32, isOutput=False)
    lam_d = nc.declare_dram_parameter("lam_ct", [P, 8], f32, isOutput=False)

    out_d = nc.declare_dram_parameter("out", [1024, D], f32, isOutput=True)

    with tile.TileContext(nc) as tc:
        with tc.tile_pool(name="glob", bufs=1) as glob:
            MW = W_SCAN + 256  # group-0 width (warmup + chunks 1-2)
            lam_sb = glob.tile([P, 8], f32, name="lam_sb")
            mask_rep = glob.tile([P, MW], f32, name="mask_rep")
            eps_sb = glob.tile([P, 1], f32, name="eps_sb")
            nc.vector.memset(eps_sb[:], 1e-5)
            ident = glob.tile([P, P], f32, name="ident")
            make_identity(nc, ident)
            ident_b = glob.tile([P, P], bf16, name="ident_b")
            nc.scalar.copy(ident_b[:], ident[:])
            io0 = glob.tile([P, 1], f32, name="io0")
            io1 = glob.tile([P, 1], f32, name="io1")
            nc.gpsimd.iota(io0[:], pattern=[[0, 1]], base=0, channel_multiplier=1,
                           allow_small_or_imprecise_dtypes=True)
            nc.gpsimd.iota(io1[:], pattern=[[0, 1]], base=128, channel_multiplier=1,
                           allow_small_or_imprecise_dtypes=True)
            convb_sb = None
            if not convb0:
                convb_sb = glob.tile([P, 8], f32, name="convb_sb")
                nc.sync.dma_start(convb_sb[:], convb_d[:])
            bb2_sb = None
            if not bb20:
                bb2_sb = glob.tile([P, 8], f32, name="bb2_sb")
                nc.sync.dma_start(bb2_sb[:], bb2_d[:])
            downb_rep = None
            if not downb0:
                downb_rep = glob.tile([P, D], f32, name="downb_rep")
                nc.sync.dma_start(downb_rep[:],
                                  downb_d[:][None, :].to_broadcast([P, D]))
            slnw_rep = None
            if not slnw1:
                slnw_rep = glob.tile([P, D], f32, name="slnw_rep")
                nc.sync.dma_start(slnw_rep[:],
                                  slnw_d[:][None, :].to_broadcast([P, D]))

            # persistent across phases
            z_all = glob.tile([P, N_CH, D], bf16, name="z_all")
            # hsT/bw in fp8e4: the b-proj runs as DoubleRow matmuls (K=256 per
            # instruction, ec-pairs adjacent in both layouts). z stays bf16 for
            # the residual; only the b-proj input is quantized (~1e-2 end2end,
            # gate is 2e-2).
            hsT = glob.tile([P, 8, S_LOC], f8, name="hsT")
            h_all = glob.tile([P, 8, S_LOC], bf16, name="h_all")
            bw_sb = glob.tile([P, 8, D], f8, name="bw_sb")
            # first e-half of the c-proj weights, prefetched late in phase A
            # (the full 2.1MB does not fit alongside the phase-A pools)
            cw_h0 = glob.tile([P, 8, 512], bf16, name="cw_h0")

            def ln_from_psums(pools, ps0, ps1_, z_out, rows):
                """LayerNorm over D from two [rows,512] psum halves -> z (bf16)."""
                p1t = pools
                stats = p1t.tile([P, 2, 6], f32, name="stats", bufs=2)
                nc.vector.bn_stats(out=stats[0:rows, 0, :], in_=ps0)
                nc.vector.bn_stats(out=stats[0:rows, 1, :], in_=ps1_)
                mv = p1t.tile([P, 2], f32, name="mv", bufs=2)
                nc.vector.bn_aggr(out=mv[0:rows, :], in_=stats[0:rows, :, :])
                rstd = p1t.tile([P, 1], f32, name="rstd", bufs=2)
                nc.scalar.activation(rstd[0:rows, :], mv[0:rows, 1:2], AF.Sqrt,
                                     bias=eps_sb[0:rows, :])
                nc.vector.reciprocal(rstd[0:rows, :], rstd[0:rows, :])
                nc.vector.tensor_scalar(out=z_out[0:rows, 0:512], in0=ps0,
                                        scalar1=mv[0:rows, 0:1],
                                        scalar2=rstd[0:rows, :],
                                        op0=OP.subtract, op1=OP.mult)
                nc.vector.tensor_scalar(out=z_out[0:rows, 512:1024], in0=ps1_,
                                        scalar1=mv[0:rows, 0:1],
                                        scalar2=rstd[0:rows, :],
                                        op0=OP.subtract, op1=OP.mult)
                if not slnw1:
                    nc.gpsimd.tensor_tensor(out=z_out[0:rows, :],
                                            in0=z_out[0:rows, :],
                                            in1=slnw_rep[0:rows, :], op=OP.mult)

            def bproj_scan(p1t, psb_pool, col0, W):
                for dc in range(8):
                    psb = psb_pool.tile([P, 512], f32, name="psb", tag="psb")
                    for ecp in range(4):
                        nc.tensor.matmul(
                            psb[:, :W],
                            bw_sb[:, 2 * ecp: 2 * ecp + 2, dc * 128:(dc + 1) * 128],
                            hsT[:, 2 * ecp: 2 * ecp + 2, col0: col0 + W],
                            start=(ecp == 0), stop=(ecp == 3),
                            perf_mode=DR)
                    data1 = psb[:, :W]
                    if (col0 == 0) or (not bb20):
                        vals = p1t.tile([P, 512], f32, name="vals", bufs=2)
                        if not bb20:
                            nc.vector.tensor_scalar(out=vals[:, :W], in0=psb[:, :W],
                                                    scalar1=bb2_sb[:, dc:dc + 1],
                                                    scalar2=None, op0=OP.add)
                            if col0 == 0:
                                nc.vector.tensor_tensor(
                                    out=vals[:, :W], in0=vals[:, :W],
                                    in1=mask_rep[:, :W], op=OP.mult)
                        else:
                            nc.vector.tensor_tensor(
                                out=vals[:, :W], in0=psb[:, :W],
                                in1=mask_rep[:, :W], op=OP.mult)
                        data1 = vals[:, :W]
                    init = (0.0 if col0 == 0
                            else h_all[:, dc, col0 - 1: col0])
                    nc.vector.tensor_tensor_scan(
                        out=h_all[:, dc, col0: col0 + W],
                        data0=lam_sb[:, dc:dc + 1].to_broadcast([P, W]),
                        data1=data1,
                        initial=init, op0=OP.mult, op1=OP.add)

            # ---------------- Phase A: fused stem + down-conv + LN + z^T ----
            with tc.tile_pool(name="paw", bufs=1) as paw, \
                 tc.tile_pool(name="pat", bufs=2) as pat, \
                 tc.tile_pool(name="ps1", bufs=2, space="PSUM") as ps1, \
                 tc.tile_pool(name="psd", bufs=2, space="PSUM") as psd, \
                 tc.tile_pool(name="pst", bufs=2, space="PSUM") as pst, \
                 tc.tile_pool(name="psb", bufs=2, space="PSUM") as psb_pool:
                stem_sb0 = paw.tile([P, 2, N_TAPS, 128], bf16, name="stem_sb0")
                stem_sb1 = paw.tile([P, 2, N_TAPS, 128], bf16, name="stem_sb1")
                stem_sbs = (stem_sb0, stem_sb1)
                dw_sb = paw.tile([P, 4, 8, D], bf16, name="dw_sb")

                x_reps, ohs = {}, {}

                def issue_xrep(tt):
                    x_rep = pat.tile([P, 520], bf16, name="x_rep", bufs=3)
                    nc.sync.dma_start(x_rep[:], x_d[tt])
                    x_reps[tt] = x_rep

                # latency-critical first: x for tile 0, then the half-0 stem
                # tables (the first CC_ORDER blocks read only half 0)
                issue_xrep(0)
                nc.sync.dma_start(stem_sb0[:, 0], stem_d[0][:, 0])
                def build_oh(tt):
                    w8 = tile_cols(tt) + 8
                    x_rep = x_reps.pop(tt)
                    oh = pat.tile([P, 2, 520], bf16, name="oh", bufs=3)
                    nc.vector.tensor_scalar(out=oh[:, 0, 0:w8],
                                            in0=x_rep[:, 0:w8],
                                            scalar1=io0[:], scalar2=None,
                                            op0=OP.is_equal)
                    nc.vector.tensor_scalar(out=oh[:, 1, 0:w8],
                                            in0=x_rep[:, 0:w8],
                                            scalar1=io1[:], scalar2=None,
                                            op0=OP.is_equal)
                    ohs[tt] = oh

                hm_ts = {}

                def stem(tt):
                    cols = tile_cols(tt)
                    oh = ohs.pop(tt)
                    hm_t = pat.tile([P, 8, 512], bf16, name="hm_t", bufs=3)
                    hm_ts[tt] = hm_t
                    for cc in CC_ORDER:
                        ci, half = cc // 2, cc % 2
                        taps = TAPS_OF_CONV[ci]
                        ps = ps1.tile([P, 512], f32, name="ps", tag="ps")
                        n_mm = len(taps) * 2
                        i = 0
                        for vc in range(2):
                            for kk in taps:
                                off = TAPS[kk][2]
                                nc.tensor.matmul(
                                    ps[:, 0:cols],
                                    stem_sbs[vc][:, half, kk, :],
                                    oh[:, vc, 4 + off: 4 + off + cols],
                                    start=(i == 0), stop=(i == n_mm - 1))
                                i += 1
                        bias = 0.0 if convb0 else convb_sb[:, cc:cc + 1]
                        nc.scalar.activation(hm_t[:, cc, 0:cols], ps[:, 0:cols],
                                             AF.Gelu, bias=bias)

                def down_ln_warm(tt):
                    """Warmup-tile down-conv in [e, s] orientation (32-row
                    matmuls), then PE-transpose back for the standard LN."""
                    S = tile_cols(tt) // 4          # 32 scan steps
                    hm_sb = hm_ts.pop(tt)
                    hdT = pat.tile([P, 8 * S], bf16, name="hdT", bufs=1)
                    for ec in range(8):
                        psw_t = psd.tile([P, 512], f32, name="psdt", tag="psdt")
                        psw = psw_t[:, 0:S]
                        i = 0
                        for j in range(4):
                            for dc in range(8):
                                nc.tensor.matmul(
                                    psw[:],
                                    dw_sb[:, j, dc, ec * 128:(ec + 1) * 128],
                                    hm_sb[:, dc, j:4 * S:4],
                                    start=(i == 0), stop=(i == 31))
                                i += 1
                        nc.scalar.copy(hdT[:, ec * S:(ec + 1) * S], psw[:])
                    hd0 = pat.tile([P, D], bf16, name="hd0", bufs=1)
                    for ec in range(8):
                        pt = pst.tile([P, P], bf16, name="pt", tag="pt")
                        nc.tensor.transpose(
                            pt[0:S, :], hdT[:, ec * S:(ec + 1) * S],
                            ident_b[:])
                        nc.scalar.copy(hd0[0:S, ec * 128:(ec + 1) * 128],
                                       pt[0:S, :])
                    p0, p1_ = hd0[0:S, 0:512], hd0[0:S, 512:1024]
                    if not downb0:
                        t0 = pat.tile([P, D], f32, name="hd_t", bufs=2)
                        nc.vector.tensor_tensor(out=t0[0:S, :], in0=hd0[0:S, :],
                                                in1=downb_rep[0:S, :], op=OP.add)
                        p0, p1_ = t0[0:S, 0:512], t0[0:S, 512:1024]
                    ln_from_psums(pat, p0, p1_, z_all[:, tt, :], S)

                def down_ln(tt):
                    if tt == 0:
                        return down_ln_warm(tt)
                    cols = tile_cols(tt)
                    rows = cols // 4
                    hm_sb = hm_ts.pop(tt)
                    pss = []
                    for eh in range(2):
                        ps = psd.tile([P, 512], f32, name="psdt", tag="psdt")
                        i = 0
                        for j in range(4):
                            for dc in range(8):
                                nc.tensor.matmul(
                                    ps[0:rows, :],
                                    hm_sb[:, dc, j:cols:4],
                                    dw_sb[:, j, dc, eh * 512:(eh + 1) * 512],
                                    start=(i == 0), stop=(i == 31))
                                i += 1
                        pss.append(ps)
                    p0, p1_ = pss[0][0:rows, :], pss[1][0:rows, :]
                    if not downb0:
                        t0 = pat.tile([P, D], f32, name="hd_t", bufs=2)
                        nc.vector.tensor_tensor(out=t0[0:rows, 0:512], in0=p0,
                                                in1=downb_rep[0:rows, 0:512],
                                                op=OP.add)
                        nc.vector.tensor_tensor(out=t0[0:rows, 512:1024], in0=p1_,
                                                in1=downb_rep[0:rows, 512:1024],
                                                op=OP.add)
                        p0, p1_ = t0[0:rows, 0:512], t0[0:rows, 512:1024]
                    ln_from_psums(pat, p0, p1_, z_all[:, tt, :], rows)

                def transp(tt):
                    rows = tile_cols(tt) // 4
                    col = chunk_col(tt)
                    for ec in range(8):
                        pt = pst.tile([P, P], bf16, name="pt", tag="pt")
                        nc.tensor.transpose(
                            pt[:, 0:rows],
                            z_all[0:rows, tt, ec * 128:(ec + 1) * 128],
                            ident_b[0:rows, 0:rows])
                        nc.scalar.copy(hsT[:, ec, col: col + rows], pt[:, 0:rows])

                # -- emission: 3-tile software pipeline --
                nc.sync.dma_start(stem_sb1[:, 0], stem_d[1][:, 0])
                issue_xrep(1)
                nc.sync.dma_start(stem_sb0[:, 1], stem_d[0][:, 1])
                nc.sync.dma_start(stem_sb1[:, 1], stem_d[1][:, 1])
                for tt in (2, 3, 4):
                    issue_xrep(tt)
                # small params deferred behind the latency-critical loads
                nc.sync.dma_start(lam_sb[:], lam_d[:])
                nc.sync.dma_start(mask_rep[:],
                                  mask_d[0:MW][None, :].to_broadcast([P, MW]))
                # bulk weight loads: issued before any consumer instruction
                for j in range(4):
                    for k in range(2):
                        nc.sync.dma_start(dw_sb[:, j, k * 4:(k + 1) * 4, :],
                                          dw_d[:, j, k * 4:(k + 1) * 4, :])
                nc.sync.dma_start(bw_sb[:], bw_d[:])
                build_oh(0)
                stem(0)
                build_oh(1)
                stem(1)
                build_oh(2)
                stem(2)
                for t in range(N_TT):
                    if t + 5 < N_TT:
                        issue_xrep(t + 5)
                    if t + 3 < N_TT:
                        build_oh(t + 3)
                    down_ln(t)
                    if t + 3 < N_TT:
                        stem(t + 3)
                    if t >= 1:
                        transp(t - 1)
                    if t == 3:
                        bproj_scan(pat, psb_pool, *GROUPS[0])
                    if t == 6:
                        bproj_scan(pat, psb_pool, *GROUPS[1])
                    if t == 7:
                        nc.sync.dma_start(cw_h0[:], cw_d[:, :, 0:512])
                transp(N_TT - 1)
                bproj_scan(pat, psb_pool, *GROUPS[2])

            # ---------------- Phase C: c-proj + residual + final LN ---------
            with tc.tile_pool(name="pcw", bufs=1) as pcw, \
                 tc.tile_pool(name="pct", bufs=2) as pct, \
                 tc.tile_pool(name="psc", bufs=3, space="PSUM") as psc_pool:
                cw_h1 = pcw.tile([P, 8, 512], bf16, name="cw_h1")
                nc.sync.dma_start(cw_h1[:], cw_d[:, :, 512:1024])
                cw_halves = (cw_h0, cw_h1)
                slncb_rep = None
                if not slncb0:
                    slncb_rep = pcw.tile([P, D], f32, name="slncb_rep")
                    nc.sync.dma_start(slncb_rep[:],
                                      slncb_d[:][None, :].to_broadcast([P, D]))
                lruw_rep = lrub_rep = None
                if not lruw1:
                    lruw_rep = pcw.tile([P, D], f32, name="lruw_rep")
                    nc.sync.dma_start(lruw_rep[:],
                                      lruw_d[:][None, :].to_broadcast([P, D]))
                if not lrub0:
                    lrub_rep = pcw.tile([P, D], f32, name="lrub_rep")
                    nc.sync.dma_start(lrub_rep[:],
                                      lrub_d[:][None, :].to_broadcast([P, D]))

                for c in range(1, N_CH):
                    col = chunk_col(c)
                    res = pct.tile([P, D], f32, name="res", bufs=2)
                    for eh in range(2):
                        psc = psc_pool.tile([P, 512], f32, name="psc", tag="psc")
                        for dc in range(8):
                            nc.tensor.matmul(
                                psc[:],
                                h_all[:, dc, col: col + 128],
                                cw_halves[eh][:, dc, :],
                                start=(dc == 0), stop=(dc == 7))
                        nc.vector.tensor_tensor(
                            out=res[:, eh * 512:(eh + 1) * 512],
                            in0=psc[:],
                            in1=z_all[:, c, eh * 512:(eh + 1) * 512],
                            op=OP.add)
                    if not slncb0:
                        nc.gpsimd.tensor_tensor(out=res[:], in0=res[:],
                                                in1=slncb_rep[:], op=OP.add)
                    stats2 = pct.tile([P, 2, 6], f32, name="stats2", bufs=2)
                    res_g = res[:].rearrange("p (g f) -> p g f", g=2)
                    nc.vector.bn_stats(out=stats2[:, 0, :], in_=res_g[:, 0, :])
                    nc.vector.bn_stats(out=stats2[:, 1, :], in_=res_g[:, 1, :])
                    mv2 = pct.tile([P, 2], f32, name="mv2", bufs=2)
                    nc.vector.bn_aggr(out=mv2[:], in_=stats2[:])
                    rstd2 = pct.tile([P, 1], f32, name="rstd2", bufs=2)
                    nc.scalar.activation(rstd2[:], mv2[:, 1:2], AF.Sqrt,
                                         bias=eps_sb[:])
                    nc.vector.reciprocal(rstd2[:], rstd2[:])
                    o_t = pct.tile([P, D], f32, name="o_t", bufs=2)
                    nc.vector.tensor_scalar(out=o_t[:], in0=res[:],
                                            scalar1=mv2[:, 0:1],
                                            scalar2=rstd2[:],
                                            op0=OP.subtract, op1=OP.mult)
                    if not lruw1:
                        nc.gpsimd.tensor_tensor(out=o_t[:], in0=o_t[:],
                                                in1=lruw_rep[:], op=OP.mult)
                    if not lrub0:
                        nc.gpsimd.tensor_tensor(out=o_t[:], in0=o_t[:],
                                                in1=lrub_rep[:], op=OP.add)
                    nc.sync.dma_start(out_d[(c - 1) * 128: c * 128, :], o_t[:])

    nc.finalize()
    return nc


def _prep_host(inputs):
    f = np.float32
    import ml_dtypes
    bf = ml_dtypes.bfloat16
    embed = np.asarray(inputs["embed"], f)
    conv_ws = [np.asarray(inputs[k], f) for k in
               ("conv1_w", "conv2_w", "conv4_w", "conv8_w")]
    conv_bs = [np.asarray(inputs[k], f) for k in
               ("conv1_b", "conv2_b", "conv4_b", "conv8_b")]
    down_w = np.asarray(inputs["down_w"], f)
    log_lam = np.asarray(inputs["log_lambda_raw"], f)
    lam = (1.0 / (1.0 + np.exp(-log_lam.astype(np.float64)))).astype(f)
    b_w = np.asarray(inputs["b_w"], f)
    c_w = np.asarray(inputs["c_w"], f)
    slnw = np.asarray(inputs["stem_ln_w"], f)
    slnb = np.asarray(inputs["stem_ln_b"], f)
    lruw = np.asarray(inputs["lru_ln_w"], f)
    lrub = np.asarray(inputs["lru_ln_b"], f)
    c_b = np.asarray(inputs["c_b"], f)
    down_b = np.asarray(inputs["down_b"], f)

    stem_w = np.empty((2, P, 2, N_TAPS, 128), bf)
    for kk, (ci, j, _off) in enumerate(TAPS):
        fused = embed @ conv_ws[ci][:, :, j].T        # [256v, 256c]
        stem_w[:, :, :, kk, :] = fused.reshape(2, P, 2, 128).astype(bf)
    convb = np.concatenate(conv_bs).reshape(8, P).T.copy()      # [p, cc]

    down_wt = (down_w.transpose(1, 2, 0)                        # [d, j, e]
               .reshape(8, P, 4, D).transpose(1, 2, 0, 3)
               .astype(bf))                                     # [p, j, dc, e]
    one_m = (1.0 - lam)
    # values[d,t] = sum_e [(1-lam_d) b_w[d,e]] zs^T[e,t] + (1-lam)(b_w@slnb + b_b)
    # (zs = z*slnw is materialized on-device; slnb folds into bb2)
    b_wt = ((b_w.T * one_m[None, :])                            # [e, d]
            .reshape(8, P, D).transpose(1, 0, 2)
            .astype(ml_dtypes.float8_e4m3fn))                   # [p, ec, d]
    bb2 = (one_m * (b_w @ slnb + np.asarray(inputs["b_b"], f))
           ).reshape(8, P).T.copy()
    c_wt = c_w.T.reshape(8, P, D).transpose(1, 0, 2).astype(bf)  # [p, dc, e]
    lam_ct = lam.reshape(8, P).T.copy()
    slncb = slnb + c_b

    flags = (
        bool(np.all(convb == 0.0)),
        bool(np.all(down_b == 0.0)),
        bool(np.all(slnw == 1.0)),
        bool(np.all(bb2 == 0.0)),
        bool(np.all(slncb == 0.0)),
        bool(np.all(lruw == 1.0)),
        bool(np.all(lrub == 0.0)),
    )

    shared = dict(
        stem_w=stem_w, convb=convb, down_wt=down_wt, down_b=down_b,
        b_wt=b_wt, bb2=bb2, c_wt=c_wt, slncb=slncb,
        slnw=slnw, lruw=lruw, lrub=lrub, lam_ct=lam_ct,
    )

    x = np.asarray(inputs["x"]).astype(np.int64)
    in_maps = []
    for core in range(8):
        b, h = core // 2, core % 2
        t0 = h * 4096
        idx = t0 - (4 * W_SCAN + 4) + np.arange(X_LOC)
        valid = (idx >= 0) & (idx < T)
        x_loc = np.full((X_LOC,), SENTINEL, bf)
        x_loc[valid] = x[b, idx[valid]].astype(bf)
        # pre-broadcast per tile on host: device-side 128-way broadcast DMAs
        # are far slower than a contiguous copy
        x_rep = np.full((N_TT, P, 520), SENTINEL, bf)
        for tt in range(N_TT):
            w8 = tile_cols(tt) + 8
            x0_ = tile_x0(tt)
            x_rep[tt, :, 0:w8] = x_loc[x0_: x0_ + w8][None, :]
        mask = np.ones((S_LOC,), f)
        if h == 0:
            mask[:W_SCAN] = 0.0
        m = dict(shared)
        m["x_rep"] = x_rep
        m["mask"] = mask
        in_maps.append(m)
    return in_maps, flags


def kernel(**inputs) -> np.ndarray:
    in_maps, flags = _prep_host(inputs)
    if flags not in _CACHE:
        _CACHE[flags] = _build(flags)
    nc = _CACHE[flags]
    res = run_bass_kernel_spmd(nc, in_maps, list(range(8)))
    out = np.empty((B, 2048, D), np.float32)
    for core in range(8):
        b, h = core // 2, core % 2
        out[b, h * 1024:(h + 1) * 1024, :] = res.results[core]["out"]
    return out
